# revision 1
# baseline (speedup 1.0000x reference)
"""DFA scan kernel for Trainium2 (8 NeuronCores).

Problem: q_{t+1} = delta[seq_t] @ q_t over 524288 symbols; answer = f . q_final.

Strategy (sequence parallelism over the monoid of n x n maps, per the
sharding hint, applied to a suffix window with a rigorous certificate):

  The transition matrices are column-stochastic.  The full answer is
  f^T (D_L ... D_1) q0.  Split the product as  f^T M_tail M_prefix q0.
  M_prefix q0 is *some* probability vector p (exactly, in real
  arithmetic).  So answer = r . p with r = f^T M_tail, and therefore
  answer is bounded between min(r) and max(r) REGARDLESS of the prefix.
  If max(r) - min(r) is tiny relative to |r|, the suffix product alone
  determines the answer to that tolerance - a certificate with no
  distributional assumption.  For random normalized-uniform delta the
  per-symbol contraction is |lambda_2| ~ 0.07, so a 2048-symbol suffix
  leaves a spread of ~1e-9 relative: the certificate always holds.
  If it does not hold (adversarial inputs), we fall back to an exact
  CPU evaluation - slow but correct for any input.

  M_tail (2048 symbols) is computed on 8 NeuronCores: core c takes a
  contiguous 256-symbol sub-chunk and tree-reduces its 256 matrices
  with 255 64x64x64 fp32 matmuls on the tensor engine; the 8 chunk
  maps are multiplied on the host (7 tiny matmuls).

  Tree trick to avoid on-chip transposes: matmul computes lhsT.T @ rhs.
  Store each tree node's product P either naturally or transposed:
  node n at any level outputs natural form iff n is even.  A parent
  combining (earlier child A natural, later child B stored transposed)
  gets   natural:    R@L  = matmul(lhsT=B, rhs=A)
         transposed: (R@L)^T = matmul(lhsT=A, rhs=B)
  so every node costs exactly one matmul and children are always in
  the required forms by induction.

  SBUF placement: node n's result lives at partition home
  64*((n>>3)&1), column block (n>>4)*8 + (n&7) of its level buffer.
  All 4 matmuls of one PSUM group then share one PE row-group (their
  concurrent drains never target the same PSUM bank from different
  row-groups - that combination faults the device), and consecutive
  groups alternate row-groups so LDWEIGHTS overlaps in-flight MMs.
  Leaves (gathered delta[sym] / delta[sym]^T) are pre-placed by the
  host in the same pattern, so the device only does matmuls + PSUM->
  SBUF copies + DMA.
"""

import numpy as np

N = 64
NSYM = 128
NCORES = 8
T_LEAVES = 64                # leaves (symbols) per core
K_TAIL = T_LEAVES * NCORES   # suffix window length
CERT_RTOL = 3e-4             # certificate: spread(r) <= CERT_RTOL * scale(r)
LINEARIZE = False

_cache = {}


def _leaf_layout(T):
    i = np.arange(T)
    home = (i >> 3) & 1
    col = (i >> 4) * 8 + (i & 7)
    return home, col


def _level_width_cols(n_nodes):
    # column blocks needed by the placement rule at one level
    return (n_nodes // 2) if n_nodes >= 16 else n_nodes


def _build_nc(T):
    """Build the SPMD Bass program: tree-reduce T gathered 64x64 matrices."""
    import concourse.tile as tile
    from concourse import bacc, mybir
    from contextlib import ExitStack

    f32 = mybir.dt.float32
    W = 64 * _level_width_cols(T)  # leaf buffer free width (elements)

    nc = bacc.Bacc("TRN2", target_bir_lowering=False, debug=False,
                   num_devices=NCORES)
    SLAB_W = 512 if W % 512 == 0 else W
    n_slabs = W // SLAB_W
    leaves_d = nc.dram_tensor("leaves", [n_slabs, 128, SLAB_W], f32,
                              kind="ExternalInput")
    out_d = nc.dram_tensor("out", [N, N], f32, kind="ExternalOutput")

    with tile.TileContext(nc, linearize=LINEARIZE) as tc, ExitStack() as ctx:
        sb = ctx.enter_context(tc.tile_pool(name="sb", bufs=1))
        ps = ctx.enter_context(tc.tile_pool(name="ps", bufs=8, space="PSUM"))

        leaf_t = sb.tile([128, W], f32, tag="leaves")
        for s in range(n_slabs):
            nc.sync.dma_start(out=leaf_t[:, s * SLAB_W:(s + 1) * SLAB_W],
                              in_=leaves_d[s, :, :])

        def emit_group(g, psum_t, src):
            # 4 matmuls of group g into psum_t[outhome:outhome+64, :]
            outhome = 64 * (((g * 4) >> 3) & 1)
            for j in range(4):
                n = g * 4 + j
                srchome = 64 * ((n >> 2) & 1)
                c2n = ((n >> 3) * 8 + 2 * (n & 3)) * 64
                A = src[srchome:srchome + 64, c2n:c2n + 64]
                B = src[srchome:srchome + 64, c2n + 64:c2n + 128]
                o = psum_t[outhome:outhome + 64, j * 64:(j + 1) * 64]
                if n % 2 == 0:
                    nc.tensor.matmul(o, lhsT=B, rhs=A, start=True, stop=True)
                else:
                    nc.tensor.matmul(o, lhsT=A, rhs=B, start=True, stop=True)

        src, n_src = leaf_t, T
        level = 0
        while n_src > 1:
            nn = n_src // 2  # nodes at this level
            dst = sb.tile([128, 64 * _level_width_cols(nn)], f32,
                          tag=f"lvl{level}")
            if nn >= 16:
                # quads of groups: (4a,4a+2) share a psum tile (homes 0/64
                # at identical dst cols), likewise (4a+1,4a+3); one
                # [128,256] copy drains each pair.
                for a in range(nn // 16):
                    pe_t = ps.tile([128, 256], f32, tag="ps")
                    po_t = ps.tile([128, 256], f32, tag="ps")
                    for q in range(4):
                        emit_group(4 * a + q, (pe_t, po_t)[q & 1], src)
                    nc.vector.tensor_copy(
                        dst[:, a * 512:a * 512 + 256], pe_t[:, :])
                    nc.vector.tensor_copy(
                        dst[:, a * 512 + 256:a * 512 + 512], po_t[:, :])
            elif nn >= 4:
                for g in range(nn // 4):
                    psum_t = ps.tile([128, 256], f32, tag="ps")
                    emit_group(g, psum_t, src)
                    dstcol = ((g * 4) >> 4) * 512 + ((g * 4) & 7) * 64
                    nc.vector.tensor_copy(
                        dst[0:64, dstcol:dstcol + 256], psum_t[0:64, :])
            else:
                psum_t = ps.tile([128, 64 * nn], f32, tag="ps")
                for n in range(nn):
                    A = src[0:64, 2 * n * 64:2 * n * 64 + 64]
                    B = src[0:64, (2 * n + 1) * 64:(2 * n + 1) * 64 + 64]
                    o = psum_t[0:64, n * 64:(n + 1) * 64]
                    if n % 2 == 0:
                        nc.tensor.matmul(o, lhsT=B, rhs=A, start=True, stop=True)
                    else:
                        nc.tensor.matmul(o, lhsT=A, rhs=B, start=True, stop=True)
                nc.vector.tensor_copy(dst[0:64, 0:64 * nn], psum_t[0:64, 0:64 * nn])
            src, n_src = dst, nn
            level += 1

        nc.sync.dma_start(out=out_d[:, :], in_=src[0:64, 0:64])
    nc.compile()
    return nc


def _build_leaf_arrays(delta, tail_syms):
    """Host-side gather: per-core (128, W) leaf buffers in tree placement."""
    T = T_LEAVES
    home, col = _leaf_layout(T)
    W = 64 * _level_width_cols(T)
    deltaT = np.ascontiguousarray(np.swapaxes(delta, 1, 2))
    bufs = []
    for c in range(NCORES):
        syms = tail_syms[c * T:(c + 1) * T]
        vals = delta[syms].copy()          # (T, 64, 64) natural
        vals[1::2] = deltaT[syms[1::2]]    # odd leaves transposed
        lb = np.zeros((128, W), np.float32)
        lb4 = lb.reshape(2, 64, W // 64, 64)
        lb4[home, :, col, :] = vals
        slab_w = 512 if W % 512 == 0 else W
        slabs = np.ascontiguousarray(
            lb.reshape(128, W // slab_w, slab_w).swapaxes(0, 1))
        bufs.append(slabs)
    return bufs


def _cpu_exact(delta, f, seq):
    """Unconditional fallback: exact sequential scan on the host."""
    n = delta.shape[1]
    q = np.zeros(n, np.float32)
    q[0] = 1.0
    d = np.asarray(delta, np.float32)
    for s in np.asarray(seq):
        q = d[s] @ q
    return np.asarray(np.float32(q @ np.asarray(f, np.float32)))


def kernel(delta, f, seq):
    delta = np.ascontiguousarray(np.asarray(delta, np.float32))
    f = np.asarray(f, np.float32)
    seq = np.asarray(seq)

    if delta.shape != (NSYM, N, N) or len(seq) < K_TAIL:
        return _cpu_exact(delta, f, seq)

    from concourse.bass_utils import run_bass_kernel_spmd

    if "nc" not in _cache:
        _cache["nc"] = _build_nc(T_LEAVES)
    nc = _cache["nc"]

    tail = np.asarray(seq[-K_TAIL:], np.int64)
    in_maps = [{"leaves": lb} for lb in _build_leaf_arrays(delta, tail)]
    results = run_bass_kernel_spmd(nc, in_maps, list(range(NCORES))).results
    maps = [np.asarray(results[c]["out"], np.float32) for c in range(NCORES)]

    M = maps[0]
    for c in range(1, NCORES):
        M = maps[c] @ M           # later chunks multiply on the left
    r = f @ M                     # answer = r . p for unknown prob vector p
    if not np.all(np.isfinite(r)):
        return _cpu_exact(delta, f, seq)
    spread = float(r.max() - r.min())
    mid = float(r.mean())
    scale = max(abs(mid), float(np.abs(r).max()))
    if spread > CERT_RTOL * max(scale, 1e-300):
        # prefix not provably forgotten -> exact fallback
        return _cpu_exact(delta, f, seq)
    return np.asarray(np.float32(mid))



# revision 2
# speedup vs baseline: 1.9786x; 1.9786x over previous
"""DFA scan kernel for Trainium2 (8 NeuronCores).

Problem: q_{t+1} = delta[seq_t] @ q_t over 524288 symbols; answer = f . q_final.

Strategy (sequence parallelism over the monoid of n x n maps, per the
sharding hint, applied to a suffix window with a rigorous certificate):

  The transition matrices are column-stochastic.  The full answer is
  f^T (D_L ... D_1) q0.  Split the product as  f^T M_tail M_prefix q0.
  M_prefix q0 is *some* probability vector p (exactly, in real
  arithmetic).  So answer = r . p with r = f^T M_tail, and therefore
  answer is bounded between min(r) and max(r) REGARDLESS of the prefix.
  If max(r) - min(r) is tiny relative to |r|, the suffix product alone
  determines the answer to that tolerance - a certificate with no
  distributional assumption.  For random normalized-uniform delta the
  per-symbol contraction is |lambda_2| ~ 0.07, so a 32-symbol suffix
  contracts the spread to ~1e-38 in exact arithmetic; the computed
  spread floors at fp32 noise (~3e-7 relative, measured), 1000x below
  the certificate threshold.  If the certificate does not hold
  (adversarial inputs), we fall back to an exact CPU evaluation -
  slow but correct for any input.

  M_tail is computed on 8 NeuronCores: core c takes a contiguous
  T-symbol sub-chunk, the host gathers its T transition matrices into
  SBUF layout, and the core tree-reduces them with T-1 64x64x64 fp32
  matmuls on the tensor engine.  The 8 chunk maps are multiplied on
  the host (7 tiny matmuls).

  Tree trick to avoid on-chip transposes: matmul computes lhsT.T @ rhs.
  Store node n's product P natural iff n is even, transposed iff odd
  (leaves included: the host pre-transposes odd leaves).  A parent
  combining children A (even, natural) and B (odd, stored transposed):
    natural:    B_later @ A_earlier = matmul(lhsT=B_stored, rhs=A)
    transposed: (B @ A)^T           = matmul(lhsT=A, rhs=B_stored)
  so every node costs exactly one matmul and children are always in
  the required forms by induction.

  All operands live on partitions 0-63; each tree level writes one
  PSUM tile and drains it with one copy back to SBUF.  Program per
  core: 1 DMA in, T-1 matmuls, log2(T) PSUM->SBUF copies, 1 DMA out.
"""

import numpy as np

N = 64
NSYM = 128
NCORES = 8
T_LEAVES = 4                 # leaves (symbols) per core
K_TAIL = T_LEAVES * NCORES   # suffix window length
CERT_RTOL = 3e-4             # certificate: spread(r) <= CERT_RTOL * scale(r)
LINEARIZE = False

_cache = {}


def _build_nc(T):
    """Build the SPMD Bass program: tree-reduce T gathered 64x64 matrices."""
    import concourse.tile as tile
    from concourse import bacc, mybir
    from contextlib import ExitStack

    f32 = mybir.dt.float32
    W = 64 * T  # leaf buffer free width (elements)

    nc = bacc.Bacc("TRN2", target_bir_lowering=False, debug=False,
                   num_devices=NCORES)
    leaves_d = nc.dram_tensor("leaves", [N, W], f32, kind="ExternalInput")
    out_d = nc.dram_tensor("out", [N, N], f32, kind="ExternalOutput")

    with tile.TileContext(nc, linearize=LINEARIZE) as tc, ExitStack() as ctx:
        sb = ctx.enter_context(tc.tile_pool(name="sb", bufs=1))
        ps = ctx.enter_context(tc.tile_pool(name="ps", bufs=8, space="PSUM"))

        leaf_t = sb.tile([128, W], f32, tag="leaves")
        nc.sync.dma_start(out=leaf_t[0:N, :], in_=leaves_d[:, :])

        src, nn, level = leaf_t, T // 2, 0
        while nn >= 1:
            psum_t = ps.tile([128, N * nn], f32, tag="ps")
            for n in range(nn):
                A = src[0:N, 2 * n * N:(2 * n + 1) * N]
                B = src[0:N, (2 * n + 1) * N:(2 * n + 2) * N]
                o = psum_t[0:N, n * N:(n + 1) * N]
                if n % 2 == 0:
                    nc.tensor.matmul(o, lhsT=B, rhs=A, start=True, stop=True)
                else:
                    nc.tensor.matmul(o, lhsT=A, rhs=B, start=True, stop=True)
            dst = sb.tile([128, N * nn], f32, tag=f"lvl{level}")
            nc.vector.tensor_copy(dst[0:N, :], psum_t[0:N, :])
            src, nn, level = dst, nn // 2, level + 1

        nc.sync.dma_start(out=out_d[:, :], in_=src[0:N, 0:N])
    nc.compile()
    return nc


def _build_leaf_arrays(delta, tail_syms, T):
    """Host-side gather: per-core (64, 64*T) leaf buffers, odd leaves ^T."""
    deltaT = np.ascontiguousarray(np.swapaxes(delta, 1, 2))
    bufs = []
    for c in range(NCORES):
        syms = tail_syms[c * T:(c + 1) * T]
        vals = delta[syms].copy()          # (T, 64, 64) natural
        vals[1::2] = deltaT[syms[1::2]]    # odd leaves transposed
        # leaf j -> cols 64j..64j+64
        lb = np.ascontiguousarray(vals.transpose(1, 0, 2).reshape(N, N * T))
        bufs.append(lb)
    return bufs


def _cpu_exact(delta, f, seq):
    """Unconditional fallback: exact sequential scan on the host."""
    n = delta.shape[1]
    q = np.zeros(n, np.float32)
    q[0] = 1.0
    d = np.asarray(delta, np.float32)
    for s in np.asarray(seq):
        q = d[s] @ q
    return np.asarray(np.float32(q @ np.asarray(f, np.float32)))


def kernel(delta, f, seq):
    delta = np.ascontiguousarray(np.asarray(delta, np.float32))
    f = np.asarray(f, np.float32)
    seq = np.asarray(seq)

    if delta.shape != (NSYM, N, N) or len(seq) < K_TAIL:
        return _cpu_exact(delta, f, seq)

    from concourse.bass_utils import run_bass_kernel_spmd

    if "nc" not in _cache:
        _cache["nc"] = _build_nc(T_LEAVES)
    nc = _cache["nc"]

    tail = np.asarray(seq[-K_TAIL:], np.int64)
    in_maps = [{"leaves": lb}
               for lb in _build_leaf_arrays(delta, tail, T_LEAVES)]
    results = run_bass_kernel_spmd(nc, in_maps, list(range(NCORES))).results
    maps = [np.asarray(results[c]["out"], np.float32) for c in range(NCORES)]

    M = maps[0]
    for c in range(1, NCORES):
        M = maps[c] @ M           # later chunks multiply on the left
    r = f @ M                     # answer = r . p for unknown prob vector p
    if not np.all(np.isfinite(r)):
        return _cpu_exact(delta, f, seq)
    spread = float(r.max() - r.min())
    mid = float(r.mean())
    scale = max(abs(mid), float(np.abs(r).max()))
    if spread > CERT_RTOL * max(scale, 1e-300):
        # prefix not provably forgotten -> exact fallback
        return _cpu_exact(delta, f, seq)
    return np.asarray(np.float32(mid))
